# revision 27
# baseline (speedup 1.0000x reference)
"""Trainium2 Bass kernel for nn_AutoCompleteDecoderModel.

Encoder LSTM (512 steps) + teacher-forced decoder LSTM (512 steps) + CE loss.

Strategy: shard the hidden dimension H=1024 across 8 NeuronCores (128 each).
Every core computes the gate slice [B=128, 512] for its H-slice each step
(full PE utilization: stationary = transposed inputs [K,128], moving = weight
columns).  The per-step hidden state exchange (each core's transposed h tile
[128, B]) goes over direct core-to-core SBUF remote DMA (SWDGE broadcasts)
with XOR slot addressing so a single SPMD program works on every core.
Weight shards are pre-permuted on the host in XOR order to match.

The decoder projection + log-softmax + masked-NLL accumulation is computed
redundantly on every core (it reads the same gathered hT tiles).  The host
sums the per-batch NLL accumulator and divides by the non-PAD count.
"""

import math
import numpy as np

A = 128      # alphabet
H = 1024     # hidden
B = 128      # batch
TC = 512     # encoder steps
TT = 513     # decoder steps + 1
PAD = 0
NCORES = 8
HS = H // NCORES          # 128
GC = 4 * HS               # 512 gate columns per core  [i f o g]
XBLK = 16                 # one-hot input steps per DMA block
TBLK = 16                 # target one-hot steps per DMA block


# ----------------------------------------------------------------------------
# host-side data prep
# ----------------------------------------------------------------------------

def _col_sel(j):
    """Gate-row selection for core j in tile-col order [i f o g] (PyTorch
    LSTM row order in the reference is i,f,g,o at offsets 0,H,2H,3H)."""
    sl = np.arange(j * HS, (j + 1) * HS)
    return np.concatenate([0 * H + sl, 1 * H + sl, 3 * H + sl, 2 * H + sl])


def _prep_core(inputs, j, g):
    f32 = np.float32
    sel = _col_sel(j)

    def src_of(k):
        return j ^ g[k]

    def phase(W_ih, W_hh, b):
        Wx = W_ih.T[:, sel] + b[None, sel]                    # [A, GC]
        Wh = np.stack([W_hh.T[src_of(k) * HS:(src_of(k) + 1) * HS, sel]
                       for k in range(NCORES)])               # [8, HS, GC]
        return np.ascontiguousarray(Wx, f32), np.ascontiguousarray(Wh, f32)

    enc_Wx, enc_Wh = phase(np.asarray(inputs["enc_W_ih"], f32),
                           np.asarray(inputs["enc_W_hh"], f32),
                           np.asarray(inputs["enc_b"], f32))
    dec_Wx, dec_Wh = phase(np.asarray(inputs["dec_W_ih"], f32),
                           np.asarray(inputs["dec_W_hh"], f32),
                           np.asarray(inputs["dec_b"], f32))
    proj_W = np.asarray(inputs["proj_W"], f32)
    projT = np.stack([np.ascontiguousarray(
        proj_W.T[src_of(k) * HS:(src_of(k) + 1) * HS, :])
        for k in range(NCORES)])                              # [8, HS, A]
    return dict(enc_Wx=enc_Wx, enc_Wh=enc_Wh, dec_Wx=dec_Wx, dec_Wh=dec_Wh,
                projT=projT)


def _onehot_blocks(idx, nsteps, nblk, blk):
    """idx [B, >=nsteps] -> [nblk, A, blk*B] one-hot-transposed blocks."""
    out = np.zeros((nblk, A, blk * B), np.float32)
    ar = np.arange(A)[:, None]
    for t in range(nsteps):
        b, s = divmod(t, blk)
        out[b, :, s * B:(s + 1) * B] = (ar == idx[:, t][None, :])
    return out


def _target_blocks(tgt, n_dec, nblk, blk):
    """Masked one-hot targets [nblk, B, blk*A] and mask [nblk, B, blk]."""
    toh = np.zeros((nblk, B, blk * A), np.float32)
    msk = np.zeros((nblk, B, blk), np.float32)
    ar = np.arange(A)[None, :]
    for t in range(n_dec):
        bi, s = divmod(t, blk)
        col = tgt[:, t + 1]
        m = (col != PAD).astype(np.float32)
        toh[bi, :, s * A:(s + 1) * A] = (ar == col[:, None]) * m[:, None]
        msk[bi, :, s] = m
    return toh, msk


# ----------------------------------------------------------------------------
# bass program
# ----------------------------------------------------------------------------

def _build(n_enc, n_dec, exchange="remote", detect_races=False):
    import concourse.bass as bass
    import concourse.bacc as bacc
    import concourse.mybir as mybir
    from contextlib import ExitStack

    f32 = mybir.dt.float32
    bf16 = mybir.dt.bfloat16
    ACTF = mybir.ActivationFunctionType
    ALU = mybir.AluOpType

    T = n_enc + n_dec
    nxblk = math.ceil(T / XBLK)
    ntblk = math.ceil(max(n_dec, 1) / TBLK)

    nc = bacc.Bacc(detect_race_conditions=detect_races)

    # ---------------- DRAM parameters ----------------
    d_encWx = nc.declare_dram_parameter("enc_Wx", [A, GC], bf16, isOutput=False)
    d_decWx = nc.declare_dram_parameter("dec_Wx", [A, GC], bf16, isOutput=False)
    d_encWh = nc.declare_dram_parameter("enc_Wh", [NCORES, HS, GC], bf16, isOutput=False)
    d_decWh = nc.declare_dram_parameter("dec_Wh", [NCORES, HS, GC], bf16, isOutput=False)
    d_projT = nc.declare_dram_parameter("projT", [NCORES, HS, A], bf16, isOutput=False)
    d_projb = nc.declare_dram_parameter("projb", [1, A], bf16, isOutput=False)
    d_ones = nc.declare_dram_parameter("ones", [1, B], bf16, isOutput=False)
    d_ident = nc.declare_dram_parameter("ident", [128, 128], bf16, isOutput=False)
    d_xoh = nc.declare_dram_parameter("xoh", [nxblk, A, XBLK * B], bf16, isOutput=False)
    d_toh = nc.declare_dram_parameter("toh", [ntblk, B, TBLK * A], f32, isOutput=False)
    d_msk = nc.declare_dram_parameter("msk", [ntblk, B, TBLK], f32, isOutput=False)
    d_nll = nc.declare_dram_parameter("nll", [B, 1], f32, isOutput=True)
    d_hfin = nc.declare_dram_parameter("hfin", [HS, B], bf16, isOutput=True)
    d_cfin = nc.declare_dram_parameter("cfin", [B, HS], f32, isOutput=True)

    ctx = ExitStack()

    # ---------------- SBUF ----------------
    def sb(name, shape, dt=f32):
        return ctx.enter_context(nc.sbuf_tensor(name, shape, dt))

    s_encWx = sb("encWx", [A, GC], bf16)
    s_decWx = sb("decWx", [A, GC], bf16)
    s_encWh = [sb(f"encWh{k}", [HS, GC], bf16) for k in range(NCORES)]
    s_decWh = [sb(f"decWh{k}", [HS, GC], bf16) for k in range(NCORES)]
    s_projT = [sb(f"projTs{k}", [HS, A], bf16) for k in range(NCORES)]
    s_projb = sb("projbs", [1, A], bf16)
    s_ones = sb("oness", [1, B], bf16)
    s_ident = sb("idents", [128, 128], bf16)
    s_xbuf = [sb(f"xbuf{i}", [A, XBLK * B], bf16) for i in range(2)]
    s_tbuf = [sb(f"tbuf{i}", [B, TBLK * A]) for i in range(2)]
    s_mbuf = [sb(f"mbuf{i}", [B, TBLK]) for i in range(2)]
    s_recv = [[sb(f"recv{i}_{k}", [128, B], bf16) for k in range(NCORES)]
              for i in range(2)]
    s_sg = [sb(f"sg{i}", [B, GC]) for i in range(2)]  # [sig_i sig_f sig_o tanh_g]
    s_thc = [sb(f"thc{i}", [B, HS]) for i in range(2)]
    s_hnew = [sb(f"hnew{i}", [B, HS], bf16) for i in range(2)]
    s_m1 = sb("m1s", [B, HS])
    s_m2 = sb("m2s", [B, HS])
    s_c = sb("cs", [B, HS])
    s_nm = [sb(f"nm{i}", [B, 1]) for i in range(2)]    # negated max
    s_se = [sb(f"se{i}", [B, 1]) for i in range(2)]    # sum exp
    s_lnse = [sb(f"lnse{i}", [B, 1]) for i in range(2)]
    s_dot = [sb(f"dot{i}", [B, 1]) for i in range(2)]
    s_t1 = sb("t1s", [B, 1])
    s_etile = [sb(f"etile{i}", [B, A]) for i in range(2)]
    s_dscr = [sb(f"dscr{i}", [B, A]) for i in range(2)]
    s_nllcols = sb("nllcols", [B, max(n_dec, 1)])
    s_nllacc = sb("nllacc", [B, 1])

    # ---------------- PSUM ----------------
    def ps(name, dt=f32):
        return ctx.enter_context(nc.psum_tensor(name, [128, 512], dt))

    p_gates = [ps("pgates0"), ps("pgates1")]
    p_tr = [ps("ptr0", bf16), ps("ptr1", bf16)]
    p_lg = [ps("plg0"), ps("plg1")]

    # ---------------- semaphores ----------------
    def sem(name):
        return ctx.enter_context(nc.semaphore(name))

    rsem = [None] + [sem(f"rs{k}") for k in range(1, NCORES)]
    lsems = [sem(f"lsem{i}") for i in range(4)]
    prep = sem("prep")
    gsem = sem("gsem")    # PE: gates group done
    lgm = sem("lgm")      # PE: logits group done
    trw = sem("trw")      # PE: transpose written
    xcons = sem("xcons")  # PE: x block consumed
    wls = sem("wls")      # sync: weights loaded
    xls = sem("xls")      # sync: x blocks loaded
    tls = sem("tls")      # sync: target blocks loaded
    ols = sem("ols")      # sync: outputs stored
    asg = sem("asg")      # ACT: sigmoid done
    grd = sem("grd")      # ACT: tanh_g done (gates bank fully read)
    athc = sem("athc")    # ACT: tanh_c done
    lfa = sem("lfa")      # ACT: exp done (logits bank read by ACT)
    aln = sem("aln")      # ACT: ln done
    dvc = sem("dvc")      # DVE: c updated
    dvh = sem("dvh")      # DVE: h_new done
    cpd = sem("cpd")      # DVE: hT copy to recv slot0 done
    d1 = sem("d1")        # DVE: logits max done
    lfd = sem("lfd")      # DVE: logits dot done (bank read by DVE)
    dvef = sem("dvef")    # DVE: loss final block done
    tcons = sem("tcons")  # DVE: toh block consumed
    dvini = sem("dvini")  # DVE: init memsets done

    NW = 2 + 2 * NCORES + NCORES + 3          # weight DMAs
    n_dec_of = lambda s: s - n_enc            # dec index of global step s

    def is_dec(s):
        return n_enc <= s < T

    def r32(ap):
        return ap

    # x tile for step t: xbuf[(t//XBLK) % 2][:, (t%XBLK)*B : ...]
    def x_ap(t):
        b, s = divmod(t, XBLK)
        return s_xbuf[b % 2][:, s * B:(s + 1) * B]

    with nc.Block() as block:

        # ------------------------------------------------ PE
        @block.tensor
        def _(pe):
            pe.wait_ge(wls, 16 * NW)
            pe.wait_ge(dvini, 1)
            for p in range(T + 1):
                par = p % 2
                if p < T:
                    # ---- gate matmuls for step p ----
                    if p % XBLK == 0:
                        pe.wait_ge(xls, 64 * (p // XBLK + 1))
                    if p >= 1:
                        pe.wait_ge(cpd, p)
                    if p >= 2:
                        pe.wait_ge(grd, p - 1)
                    Wx = s_encWx if p < n_enc else s_decWx
                    Wh = s_encWh if p < n_enc else s_decWh
                    gps = p_gates[par][:, 0:GC]
                    pe.matmul(gps, r32(x_ap(p)), r32(Wx[:, :]), start=True, stop=False)
                    pe.matmul(gps, r32(s_recv[par][0][:, :]), r32(Wh[0][:, :]),
                              start=False, stop=False)
                    for k in range(1, NCORES):
                        if p >= 1 and exchange == "remote":
                            pe.wait_ge(rsem[k], 2 * p)
                        last = k == NCORES - 1
                        mm = pe.matmul(gps, r32(s_recv[par][k][:, :]), r32(Wh[k][:, :]),
                                       start=False, stop=last)
                    mm.then_inc(gsem, 1)
                    if (p + 1) % XBLK == 0 or p == T - 1:
                        pe.sem_inc(xcons, 1)

                # ---- logits matmuls for dec step s = p-1 ----
                s = p - 1
                if is_dec(s):
                    nd = n_dec_of(s)
                    spar = s % 2
                    if nd >= 2:
                        pe.wait_ge(lfd, nd - 1)
                        pe.wait_ge(lfa, nd - 1)
                    if p == T:  # epilogue: recv tiles of parity T%2 arriving now
                        pe.wait_ge(cpd, p)
                        if exchange == "remote":
                            for k in range(1, NCORES):
                                pe.wait_ge(rsem[k], 2 * p)
                    lg = p_lg[spar][:, 0:A]
                    rpar = p % 2  # logits read h(state p) = recv parity p%2
                    pe.matmul(lg, r32(s_ones[:, :]), r32(s_projb[:, :]),
                              start=True, stop=False)
                    for k in range(NCORES):
                        last = k == NCORES - 1
                        mm = pe.matmul(lg, r32(s_recv[rpar][k][:, :]),
                                       r32(s_projT[k][:, :]), start=False, stop=last)
                    mm.then_inc(lgm, 1)

                if p < T:
                    # ---- transpose h(p+1) -> tr psum, parity (p+1)%2 ----
                    pe.wait_ge(dvh, p + 1)
                    if p >= 2:
                        pe.wait_ge(cpd, p - 1)
                    pe.transpose(p_tr[(p + 1) % 2][:, 0:B], s_hnew[par][:, :],
                                 s_ident[:, :]).then_inc(trw, 1)

        # ------------------------------------------------ ACT
        @block.scalar
        def _(act):
            for p in range(T + 1):
                par = p % 2
                if p < T:
                    act.wait_ge(gsem, p + 1)
                    if p >= 2:
                        act.wait_ge(dvh, p - 1)
                    act.activation(s_sg[par][:, 0:3 * HS], p_gates[par][:, 0:3 * HS],
                                   ACTF.Sigmoid).then_inc(asg, 1)
                    act.activation(s_sg[par][:, 3 * HS:GC], p_gates[par][:, 3 * HS:GC],
                                   ACTF.Tanh).then_inc(grd, 1)
                    act.wait_ge(dvc, p + 1)
                    act.activation(s_thc[par][:, :], s_c[:, :],
                                   ACTF.Tanh).then_inc(athc, 1)
                s = p - 1
                if is_dec(s):
                    nd = n_dec_of(s)
                    spar = s % 2
                    act.wait_ge(d1, nd + 1)
                    act.activation(s_etile[spar][:, :], p_lg[spar][:, 0:A], ACTF.Exp,
                                   bias=s_nm[spar][:, :], scale=1.0,
                                   accum_out=s_se[spar][:, :]).then_inc(lfa, 1)
                    if nd >= 2:
                        act.wait_ge(dvef, nd - 1)
                    act.drain()
                    act.activation(s_lnse[spar][:, :], s_se[spar][:, :],
                                   ACTF.Ln).then_inc(aln, 1)

        # ------------------------------------------------ DVE
        @block.vector
        def _(dve):
            for k in range(NCORES):
                dve.memset(s_recv[0][k][:, :], 0.0)
            if exchange != "remote":
                # no remote writers: slots 1..7 of parity 1 stay zero forever
                for k in range(1, NCORES):
                    dve.memset(s_recv[1][k][:, :], 0.0)
            dve.memset(s_c[:, :], 0.0).then_inc(dvini, 1)
            for p in range(T + 2):
                par = p % 2
                if p < T:
                    dve.wait_ge(asg, p + 1)
                    dve.tensor_mul(s_m1[:, :], s_sg[par][:, HS:2 * HS], s_c[:, :])
                    dve.wait_ge(grd, p + 1)
                    dve.tensor_mul(s_m2[:, :], s_sg[par][:, 0:HS], s_sg[par][:, 3 * HS:GC])
                    dve.drain()
                    dve.tensor_add(s_c[:, :], s_m1[:, :], s_m2[:, :]).then_inc(dvc, 1)
                    dve.wait_ge(athc, p + 1)
                    dve.tensor_mul(s_hnew[par][:, :], s_sg[par][:, 2 * HS:3 * HS],
                                   s_thc[par][:, :]).then_inc(dvh, 1)
                    # copy transposed h into recv slot 0, parity (p+1)%2
                    dve.wait_ge(trw, p + 1)
                    dve.tensor_copy(s_recv[(p + 1) % 2][0][:, :],
                                    p_tr[(p + 1) % 2][:, 0:B]).then_inc(cpd, 1)
                # loss stage 1 for dec step s = p-1
                s = p - 1
                if is_dec(s):
                    nd = n_dec_of(s)
                    spar = s % 2
                    dve.wait_ge(lgm, nd + 1)
                    if nd >= 2:
                        dve.wait_ge(lfa, nd - 1)   # nm WAR vs ACT exp of s-2
                        dve.wait_ge(dvef, nd - 1)  # nm/dot WAR vs stage2 of s-2
                    dve.tensor_reduce(out=s_nm[spar][:, :], in_=p_lg[spar][:, 0:A],
                                      op=ALU.max, axis=mybir.AxisListType.X,
                                      negate=True).then_inc(d1, 1)
                    ti = n_dec_of(s)
                    bi, si = divmod(ti, TBLK)
                    if si == 0:
                        dve.wait_ge(tls, 80 * (bi + 1))
                    # dot[b] = sum_a logits[b,a] * toh[b,a]
                    dve.tensor_mul(s_dscr[spar][:, :], p_lg[spar][:, 0:A],
                                   s_tbuf[bi % 2][:, si * A:(si + 1) * A])
                    dve.drain()
                    dd = dve.tensor_reduce(out=s_dot[spar][:, :],
                                           in_=s_dscr[spar][:, :],
                                           op=ALU.add, axis=mybir.AxisListType.X)
                    dd.then_inc(lfd, 1)
                # loss stage 2 for dec step s2 = p-2
                s2 = p - 2
                if is_dec(s2):
                    nd = n_dec_of(s2)
                    spar = s2 % 2
                    ti = n_dec_of(s2)
                    bi, si = divmod(ti, TBLK)
                    dve.wait_ge(aln, nd + 1)
                    if nd >= 1:
                        dve.wait_ge(dvef, nd)  # t1 WAR vs stage2 of s2-1
                    # nllcols[:, ti] = mask * (ln(se) - nm) - dot
                    dve.tensor_sub(s_t1[:, :], s_lnse[spar][:, :], s_nm[spar][:, :])
                    dve.drain()
                    dve.scalar_tensor_tensor(
                        out=s_nllcols[:, ti:ti + 1], in0=s_t1[:, :],
                        scalar=s_mbuf[bi % 2][:, si:si + 1], in1=s_dot[spar][:, :],
                        op0=ALU.mult, op1=ALU.subtract).then_inc(dvef, 1)
                    if si == TBLK - 1 or ti == n_dec - 1:
                        dve.sem_inc(tcons, 1)
            if n_dec > 0:
                dve.drain()
                dve.reduce_sum(out=s_nllacc[:, :], in_=s_nllcols[:, :],
                               axis=mybir.AxisListType.X).then_inc(dvef, 1)

        # ------------------------------------------------ GPSIMD: exchange
        @block.gpsimd
        def _(gp):
            if exchange != "remote":
                return
            for p in range(T):
                if p >= 4:
                    gp.wait_ge(lsems[p % 4], 112 * (p // 4))
                gp.wait_ge(cpd, p + 1)
                src = s_recv[(p + 1) % 2][0][:, :]
                for k in range(1, NCORES):
                    rd = [None] * NCORES
                    rd[k] = (0, k)
                    gp.remote_dma_broadcast(
                        out_ap=s_recv[(p + 1) % 2][k][:, :], in_ap=src,
                        remote_sem=rsem[k], local_sem=lsems[p % 4],
                        rdests=rd).then_inc(prep, 1)
                gp.wait_ge(prep, 7 * (p + 1))
                gp.trigger_dma(7)

        # ------------------------------------------------ SYNC: DMA
        @block.sync
        def _(sp):
            def wload(dst, src):
                sp.dma_start(out=dst, in_=src).then_inc(wls, 16)

            wload(s_encWx[:, :], d_encWx[:, :])
            wload(s_decWx[:, :], d_decWx[:, :])
            for k in range(NCORES):
                wload(s_encWh[k][:, :], d_encWh[k])
                wload(s_decWh[k][:, :], d_decWh[k])
                wload(s_projT[k][:, :], d_projT[k])
            wload(s_projb[:, :], d_projb[:, :])
            wload(s_ones[:, :], d_ones[:, :])
            wload(s_ident[:, :], d_ident[:, :])

            # interleave x / toh block loads by first-use position
            loads = [("x", b, b * XBLK) for b in range(nxblk)]
            if n_dec > 0:
                loads += [("t", b, n_enc + b * TBLK + 1) for b in range(ntblk)]
            loads.sort(key=lambda e: e[2])
            for kind, b, _pos in loads:
                if kind == "x":
                    if b >= 2:
                        sp.wait_ge(xcons, b - 1)
                    for q in range(4):
                        cs = XBLK * B // 4
                        sp.dma_start(
                            out=s_xbuf[b % 2][:, q * cs:(q + 1) * cs],
                            in_=d_xoh[b][:, q * cs:(q + 1) * cs],
                        ).then_inc(xls, 16)
                else:
                    if b >= 2:
                        sp.wait_ge(tcons, b - 1)
                    for q in range(4):
                        cs = TBLK * A // 4
                        sp.dma_start(
                            out=s_tbuf[b % 2][:, q * cs:(q + 1) * cs],
                            in_=d_toh[b][:, q * cs:(q + 1) * cs],
                        ).then_inc(tls, 16)
                    sp.dma_start(out=s_mbuf[b % 2][:, :],
                                 in_=d_msk[b]).then_inc(tls, 16)

            # outputs
            if n_dec > 0:
                sp.wait_ge(dvef, n_dec + 1)
            sp.dma_start(out=d_nll[:, :], in_=s_nllacc[:, :]).then_inc(ols, 16)
            sp.wait_ge(dvc, T)
            sp.dma_start(out=d_cfin[:, :], in_=s_c[:, :]).then_inc(ols, 16)
            sp.wait_ge(cpd, T)
            sp.dma_start(out=d_hfin[:, :],
                         in_=s_recv[T % 2][0][:, :]).then_inc(ols, 16)
            sp.wait_ge(ols, 48)

    ctx.close()
    nc.finalize()
    return nc


# ----------------------------------------------------------------------------
# entry point
# ----------------------------------------------------------------------------

def _make_in_maps(inputs, n_enc, n_dec, g=None):
    src = np.asarray(inputs["src_idx"])
    tgt = np.asarray(inputs["tgt_idx"])
    T = n_enc + n_dec
    nxblk = math.ceil(T / XBLK)
    ntblk = math.ceil(max(n_dec, 1) / TBLK)

    import ml_dtypes
    bf16 = ml_dtypes.bfloat16

    # global input sequence: enc inputs then dec inputs
    xidx = np.concatenate([src[:, :n_enc], tgt[:, :n_dec]], axis=1)
    xoh = _onehot_blocks(xidx, T, nxblk, XBLK).astype(bf16)
    toh, msk = _target_blocks(tgt, n_dec, ntblk, TBLK)

    shared = dict(
        xoh=xoh, toh=toh, msk=msk,
        projb=np.asarray(inputs["proj_b"], np.float32)[None, :].astype(bf16),
        ones=np.ones((1, B), bf16),
        ident=np.eye(128, dtype=bf16),
    )
    if g is None:
        g = list(range(NCORES))
    in_maps = []
    for j in range(NCORES):
        core = _prep_core(inputs, j, g)
        m = dict(shared)
        m.update(enc_Wx=core["enc_Wx"].astype(bf16),
                 enc_Wh=core["enc_Wh"].astype(bf16),
                 dec_Wx=core["dec_Wx"].astype(bf16),
                 dec_Wh=core["dec_Wh"].astype(bf16),
                 projT=core["projT"].astype(bf16))
        in_maps.append(m)
    return in_maps


# Effective slot->source XOR deltas of the remote-DMA broadcast routing,
# measured on this fleet (logical<->physical NC map swaps the two cross-die
# core pairs).  kernel() re-probes at runtime and falls back to this value.
G_DEFAULT = [0, 1, 2, 3, 6, 7, 4, 5]


def _build_probe():
    import concourse.bacc as bacc
    import concourse.mybir as mybir
    from contextlib import ExitStack

    bf16 = mybir.dt.bfloat16
    nc = bacc.Bacc()
    d_val = nc.declare_dram_parameter("val", [128, 128], bf16, isOutput=False)
    d_out = nc.declare_dram_parameter("rout", [NCORES, 128, 128], bf16,
                                      isOutput=True)
    ctx = ExitStack()
    s_send = ctx.enter_context(nc.sbuf_tensor("send", [128, 128], bf16))
    s_rcv = [ctx.enter_context(nc.sbuf_tensor(f"rcv{k}", [128, 128], bf16))
             for k in range(NCORES)]
    sem = ctx.enter_context(nc.semaphore("dsem"))
    rsem = [None] + [ctx.enter_context(nc.semaphore(f"prs{k}"))
                     for k in range(1, NCORES)]
    lsem = ctx.enter_context(nc.semaphore("plsem"))
    prep = ctx.enter_context(nc.semaphore("pprep"))
    cs = ctx.enter_context(nc.semaphore("pcs"))
    with nc.Block() as block:
        @block.sync
        def _(sp):
            sp.dma_start(out=s_send[:, :], in_=d_val[:, :]).then_inc(sem, 16)
            sp.wait_ge(cs, 1)
            for k in range(NCORES):
                sp.dma_start(out=d_out[k], in_=s_rcv[k][:, :]).then_inc(sem, 16)
            sp.wait_ge(sem, 16 * (NCORES + 1))

        @block.gpsimd
        def _(gp):
            gp.wait_ge(sem, 16)
            gp.dma_start(out=s_rcv[0][:, :], in_=s_send[:, :]).then_inc(sem, 16)
            for k in range(1, NCORES):
                rd = [None] * NCORES
                rd[k] = (0, k)
                gp.remote_dma_broadcast(out_ap=s_rcv[k][:, :], in_ap=s_send[:, :],
                                        remote_sem=rsem[k], local_sem=lsem,
                                        rdests=rd).then_inc(prep, 1)
            gp.wait_ge(prep, 7)
            gp.trigger_dma(7)
            for k in range(1, NCORES):
                gp.wait_ge(rsem[k], 1)
            gp.wait_ge(sem, 32)
            gp.sem_inc(cs, 1)
    ctx.close()
    nc.finalize()
    return nc


def _probe_routing():
    import ml_dtypes
    from concourse.bass_utils import run_bass_kernel_spmd

    try:
        nc = _build_probe()
        in_maps = [{"val": np.full((128, 128), float(j + 1),
                                   dtype=ml_dtypes.bfloat16)}
                   for j in range(NCORES)]
        res = run_bass_kernel_spmd(nc, in_maps, list(range(NCORES)))
        g = None
        for j in range(NCORES):
            out = res.results[j]["rout"].astype(np.float32)
            gj = []
            for k in range(NCORES):
                vals = np.unique(out[k])
                if len(vals) != 1 or vals[0] < 1:
                    return G_DEFAULT
                gj.append((int(vals[0]) - 1) ^ j)
            if g is None:
                g = gj
            elif g != gj:
                return G_DEFAULT
        if sorted(g) != list(range(NCORES)) or g[0] != 0:
            return G_DEFAULT
        return g
    except Exception:
        return G_DEFAULT


def run(inputs, n_enc=TC, n_dec=TT - 1, trace=False, g=None):
    from concourse.bass_utils import run_bass_kernel_spmd

    if g is None:
        g = _probe_routing()
    nc = _build(n_enc, n_dec)
    in_maps = _make_in_maps(inputs, n_enc, n_dec, g)
    res = run_bass_kernel_spmd(nc, in_maps, list(range(NCORES)), trace=trace)
    return nc, res


def kernel(**inputs):
    _, res = run(inputs)
    tgt = np.asarray(inputs["tgt_idx"])
    count = float((tgt[:, 1:TT] != PAD).sum())
    nll = res.results[0]["nll"].astype(np.float64)
    loss = nll.sum() / max(count, 1.0)
    return np.float32(loss)


if __name__ == "__main__":
    import reference
    inputs = reference.setup_inputs()
    print(kernel(**{k: np.asarray(v) for k, v in inputs.items()}))
